# revision 21
# baseline (speedup 1.0000x reference)
"""Trainium2 Bass kernel for nn_CIC (curve-attention GNN message passing block).

Sharding: data-parallel over batch B=16 -> 2 batches per core x 8 cores.
All weights replicated; no collectives.

v3 design notes (v1 baseline 395us, v2 311us):
  - Host casts x/curves to fp16 and uploads curves in BOTH natural [c,nl]
    and transposed [q,t,c] layouts; y written fp16, upcast on host.
  - att computed on the PE (w_att replicated across stationary cols ->
    partition-replicated PSUM), extracted via ACT row copies + reshape
    DMA + small xbar transpose.
  - Points loop split into pass A (logits+Exp), pass B1 (denominator
    matmuls + Reciprocal), pass B2 (y matmuls + epilogue) so ACT never
    thrashes its activation table.
  - v3: the two batches' phases are emitted INTERLEAVED.  Tile executes
    each engine's queue in program order, so batch-serial emission left
    the PE idle >3.4us at every softmax/extract boundary -> HAM
    re-throttled to 1.2 GHz for ~60% of the kernel (v2 trace).  The
    interleave keeps the PE queue dense; softmax/DVE/DMA work of one
    batch hides under the other batch's matmul streams.
  - v3: ci stationary narrowed from 128 to 64 columns (two 32-tile
    column groups writing PSUM partition halves) -> halves its
    LDWEIGHTS cost; zp shrinks 16.25->8.25 KB/partition.
  - v3: normalize is split in chunk halves so pass B2 starts while B1
    finishes (no PE bubble at the B1->B2 boundary).

Math (per batch b):
  att[n,l]   = sum_c w_att[c] * curves[c,n,l]
  S_l        = softmax(att, axis=l);  S_n = softmax(att, axis=n)
  curver_inter[c,n] = sum_l curves[c,n,l] * S_l[n,l]
  curves_intra[c,l] = sum_n curves[c,n,l] * S_n[n,l]
  a = wa @ curver_inter              [MID, n]
  b = wb @ curves_intra              [MID, l]
  AiT = wc.T @ a   [C, n];  AtT = wc.T @ b  [C, l]      (folded x_logits)
  logits_i = AiT.T @ x  [n, Np];  logits_t = AtT.T @ x  [l, Np]
  E = exp(logits); den = colsum(E)  (ones-matmul, replicated rows)
  UiT = a.T @ (wd[:, :MID] @ wn).T * s   [n, C]   (BN scale s folded)
  UtT = b.T @ (wd[:, MID:] @ wl).T * s   [l, C]
  y[c,p] = sum_n UiT[n,c] * E_i_norm[n,p] + sum_l UtT[l,c] * E_t_norm[l,p]
  out = leaky_relu(x + y + (beta - mean*s), 0.2)
"""

import numpy as np
import ml_dtypes

import concourse.bass as bass
import concourse.mybir as mybir
from concourse.tile import TileContext
from concourse.bass_utils import run_bass_kernel_spmd

bf16 = ml_dtypes.bfloat16
F32 = mybir.dt.float32
BF = mybir.dt.bfloat16
FP16 = mybir.dt.float16
ALU = mybir.AluOpType
AF = mybir.ActivationFunctionType
AX = mybir.AxisListType

B, C, N = 16, 256, 8192
CN, CL, MID = 128, 64, 128
NCORES = 8
NB = B // NCORES          # batches per core
NT = CN // 2              # 64 nl-tiles of 128 per batch
TG = 32                   # ci column-group size in tiles
CHW = 512                 # points chunk width
NCH = N // CHW            # 16 chunks
XJW = 2048                # x/y DMA staging width
NJ = N // XJW             # 4 staging groups
BN_EPS = 1e-5


_WS_COUNTER = [0]


def _split_excess_waits(nc, max_waits=1):
    """This walrus build rejects instructions carrying more than ~1 sem-wait
    command.  Move excess waits onto same-engine NoOps inserted right before
    the offending instruction (program order on one engine preserves the
    semantics exactly)."""
    for fn in nc.m.functions:
        for blk in fn.blocks:
            insts = list(blk.instructions)
            out = []
            changed = False
            for inst in insts:
                si = inst.sync_info
                waits = list(si.on_wait) if si and si.on_wait else []
                if len(waits) > max_waits and inst.engine is not None:
                    keep = waits[:max_waits]
                    extra = waits[max_waits:]
                    for w in extra:
                        _WS_COUNTER[0] += 1
                        nop = mybir.InstNoOp(
                            name=f"I-waitsplit-{_WS_COUNTER[0]}",
                            opcode="NoOp",
                            engine=inst.engine,
                            ins=[],
                            outs=[],
                            sync_info=mybir.SyncInfo(on_wait=[w], on_update=[]),
                        )
                        out.append(nop)
                    si.on_wait = keep
                    changed = True
                out.append(inst)
            if changed:
                blk.instructions = out


def _act_recip(nc, out_ap, in_ap):
    """ACT-engine reciprocal via raw InstActivation (the bass wrapper refuses
    Reciprocal for accuracy reasons; softmax denominators tolerate it)."""
    eng = nc.scalar
    imm = lambda v: mybir.ImmediateValue(dtype=mybir.dt.float32, value=v)
    return eng.add_instruction(
        mybir.InstActivation(
            name=nc.get_next_instruction_name(),
            func=AF.Reciprocal,
            ins=[eng.lower_ap(in_ap), imm(0.0), imm(1.0), imm(0.0)],
            outs=[eng.lower_ap(out_ap)],
        )
    )


def _build_bass():
    nc = bass.Bass()

    x_d = nc.dram_tensor("x", [NB, C, N], FP16, kind="ExternalInput")
    cn_d = nc.dram_tensor("cnat", [NB, C, N], FP16, kind="ExternalInput")
    ct_d = nc.dram_tensor("ctr", [NB, 128, NT * 256], FP16, kind="ExternalInput")
    wc_d = nc.dram_tensor("wc", [MID, C], FP16, kind="ExternalInput")
    waT_d = nc.dram_tensor("waT", [C, MID], FP16, kind="ExternalInput")
    wbT_d = nc.dram_tensor("wbT", [C, MID], FP16, kind="ExternalInput")
    wdnT_d = nc.dram_tensor("wdnT", [MID, C], FP16, kind="ExternalInput")
    wdlT_d = nc.dram_tensor("wdlT", [MID, C], FP16, kind="ExternalInput")
    wattT_d = nc.dram_tensor("wattT", [C, 128], FP16, kind="ExternalInput")
    onesn_d = nc.dram_tensor("onesn", [128, 128], BF, kind="ExternalInput")
    onesl_d = nc.dram_tensor("onesl", [64, 64], BF, kind="ExternalInput")
    i2_d = nc.dram_tensor("i2", [128, 64], FP16, kind="ExternalInput")
    eyeh_d = nc.dram_tensor("eyeh", [128, 128], FP16, kind="ExternalInput")
    betap_d = nc.dram_tensor("betap", [C, 1], F32, kind="ExternalInput")
    y_d = nc.dram_tensor("y", [NB, C, N], FP16, kind="ExternalOutput")

    from contextlib import ExitStack

    with TileContext(nc) as tc:
        with ExitStack() as _es:
            cpool = _es.enter_context(tc.tile_pool(name="const", bufs=1))
            cnpool = _es.enter_context(tc.tile_pool(name="cnat", bufs=4))
            ctpool = _es.enter_context(tc.tile_pool(name="ctrans", bufs=2))
            zppool = _es.enter_context(tc.tile_pool(name="zp", bufs=1))
            spool = _es.enter_context(tc.tile_pool(name="small", bufs=1))
            apool = _es.enter_context(tc.tile_pool(name="abuf", bufs=2))
            xpool = _es.enter_context(tc.tile_pool(name="xin", bufs=8))
            epool = _es.enter_context(tc.tile_pool(name="ebuf", bufs=1))
            rpool = _es.enter_context(tc.tile_pool(name="rbuf", bufs=1))
            arpool = _es.enter_context(tc.tile_pool(name="attrow", bufs=1))
            opool = _es.enter_context(tc.tile_pool(name="obuf", bufs=4))
            ptp = _es.enter_context(tc.tile_pool(name="pt", bufs=1, space="PSUM"))
            pap = _es.enter_context(tc.tile_pool(name="pacc", bufs=2, space="PSUM"))
            psp = _es.enter_context(tc.tile_pool(name="psmall", bufs=1, space="PSUM"))
            plp = _es.enter_context(tc.tile_pool(name="plog", bufs=2, space="PSUM"))
            pyp = _es.enter_context(tc.tile_pool(name="py", bufs=2, space="PSUM"))

            # ---- constants ----
            wc_t = cpool.tile([MID, C], FP16, tag="wc")
            nc.sync.dma_start(wc_t[:], wc_d[:])
            waT_c = []
            wbT_c = []
            wattT_c = []
            for h in range(2):
                wat = cpool.tile([128, MID], FP16, tag=f"waT{h}")
                nc.sync.dma_start(wat[:], waT_d[128 * h : 128 * (h + 1), :])
                waT_c.append(wat)
                wbt = cpool.tile([128, MID], FP16, tag=f"wbT{h}")
                nc.sync.dma_start(wbt[:], wbT_d[128 * h : 128 * (h + 1), :])
                wbT_c.append(wbt)
                wtt = cpool.tile([128, 128], FP16, tag=f"wattT{h}")
                nc.sync.dma_start(wtt[:], wattT_d[128 * h : 128 * (h + 1), :])
                wattT_c.append(wtt)
            wdnT_t = cpool.tile([MID, C], FP16, tag="wdnT")
            nc.sync.dma_start(wdnT_t[:], wdnT_d[:])
            wdlT_t = cpool.tile([MID, C], FP16, tag="wdlT")
            nc.sync.dma_start(wdlT_t[:], wdlT_d[:])
            onesn_t = cpool.tile([128, 128], BF, tag="onesn")
            nc.sync.dma_start(onesn_t[:], onesn_d[:])
            onesl_t = cpool.tile([64, 64], BF, tag="onesl")
            nc.sync.dma_start(onesl_t[:], onesl_d[:])
            i2_t = cpool.tile([128, 64], FP16, tag="i2")
            nc.sync.dma_start(i2_t[:], i2_d[:])
            eyeh_t = cpool.tile([128, 128], FP16, tag="eyeh")
            nc.sync.dma_start(eyeh_t[:], eyeh_d[:])
            # [256,1] does not fit 128 partitions; load as two chunks
            betap0 = cpool.tile([128, 1], F32, tag="betap0")
            nc.sync.dma_start(betap0[:], betap_d[0:128, :])
            betap1 = cpool.tile([128, 1], F32, tag="betap1")
            nc.sync.dma_start(betap1[:], betap_d[128:256, :])
            betaps = [betap0, betap1]

            # WD: fused ci+intra stationary.  Per 32-tile group g, tile
            # t=32g+s owns the 128-col block at flat [4096g+128s, +128).
            # Cols [0,64) hold the sparse S_l pair for ci: value S_l[2t+j, l]
            # sits at partition 64j+l, flat col 4096g + 130s + j, i.e.
            # within-block col 2s+j -- the PSUM row (n within group half) it
            # accumulates into.  Cols [64,128) hold the dense S_n diag stack
            # DT[:, t, :] for intra.  One matmul per tile computes both:
            # out rows [0,64) = ci half, rows [64,128) = intra partial.
            WD = zppool.tile([128, 2 * TG * 128], FP16, tag="WD")
            nc.gpsimd.memset(WD[:], 0.0)
            WDf = WD[:]
            WDv = WD[:].rearrange("p (g s u) -> p g s u", g=2, s=TG)

            st = [dict() for _ in range(NB)]

            def p1_load_att(b):
                s = st[b]
                # transposed curves, as two 32-tile halves (double-buffered)
                s["cth"] = []
                for hh in range(2):
                    ctt = ctpool.tile([128, TG, 256], FP16, tag="cth")
                    nc.sync.dma_start(
                        ctt[:].rearrange("p t c -> p (t c)"),
                        ct_d[b, :, 8192 * hh : 8192 * (hh + 1)],
                    )
                    s["cth"].append(ctt)
                # att matmuls from natural-layout chunks
                att_row = arpool.tile([1, N], FP16, tag="attrow")
                for j in range(NJ):
                    cnj = []
                    for cc in range(2):
                        t_ = cnpool.tile([128, XJW], FP16, tag="cnat")
                        nc.sync.dma_start(
                            t_[:],
                            cn_d[b, 128 * cc : 128 * (cc + 1), XJW * j : XJW * (j + 1)],
                        )
                        cnj.append(t_)
                    for kk in range(XJW // CHW):
                        k = j * (XJW // CHW) + kk
                        att_ps = plp.tile([128, CHW], F32, tag="pl")
                        for cc in range(2):
                            nc.tensor.matmul(
                                att_ps[:],
                                wattT_c[cc][:],
                                cnj[cc][:, CHW * kk : CHW * (kk + 1)],
                                start=(cc == 0),
                                stop=(cc == 1),
                            )
                        nc.scalar.activation(
                            att_row[:, CHW * k : CHW * (k + 1)],
                            att_ps[0:1, :],
                            AF.Copy,
                        )
                # amT[t, q] = att[128 t + q]: cross-partition reshape DMA,
                # then a small xbar transpose for attm[q, t].
                amT = apool.tile([NT, 128], FP16, tag="amT")
                nc.sync.dma_start(amT[:], att_row[:])
                attm = apool.tile([128, NT], FP16, tag="attm")
                nc.scalar.dma_start(attm[:], amT[:], transpose=True)
                s["amT"] = amT
                s["attm"] = attm

            def p2_softmax(b):
                s = st[b]
                amT, attm = s["amT"], s["attm"]
                # softmax over l (rows of amT)
                rmax = spool.tile([NT, 2], F32, tag="rmax")
                nc.vector.reduce_max(
                    rmax[:], amT[:].rearrange("t (j l) -> t j l", j=2), axis=AX.X
                )
                nmax = spool.tile([NT, 2], F32, tag="nmax")
                nc.vector.tensor_scalar_mul(nmax[:], rmax[:], -1.0)
                el = spool.tile([NT, 128], F32, tag="el")
                for j in range(2):
                    nc.scalar.activation(
                        el[:, 64 * j : 64 * (j + 1)],
                        amT[:, 64 * j : 64 * (j + 1)],
                        AF.Exp,
                        bias=nmax[:, j : j + 1],
                        scale=1.0,
                    )
                ssum = spool.tile([NT, 2], F32, tag="ssum")
                nc.vector.reduce_sum(
                    ssum[:], el[:].rearrange("t (j l) -> t j l", j=2), axis=AX.X
                )
                rsum = spool.tile([NT, 2], F32, tag="rsum")
                nc.vector.reciprocal(rsum[:], ssum[:])
                slm = spool.tile([NT, 128], FP16, tag="slm")
                for j in range(2):
                    nc.vector.tensor_scalar_mul(
                        slm[:, 64 * j : 64 * (j + 1)],
                        el[:, 64 * j : 64 * (j + 1)],
                        rsum[:, j : j + 1],
                    )
                slT_ps = ptp.tile([128, NT], FP16, tag="tps")
                nc.tensor.transpose(slT_ps[:], slm[:], eyeh_t[0:NT, 0:NT])
                slT = spool.tile([128, NT], FP16, tag="slT")
                nc.vector.tensor_copy(slT[:], slT_ps[:])

                # softmax over n (across q-halves and t)
                m1 = spool.tile([128, 1], F32, tag="m1")
                nc.vector.reduce_max(m1[:], attm[:], axis=AX.X)
                m1u = spool.tile([64, 1], F32, tag="m1u")
                nc.vector.tensor_copy(m1u[:], m1[64:128, :])
                mc = spool.tile([64, 1], F32, tag="mc")
                nc.vector.tensor_tensor(mc[:], m1[0:64, :], m1u[:], op=ALU.max)
                nmc = spool.tile([64, 1], F32, tag="nmc")
                nc.vector.tensor_scalar_mul(nmc[:], mc[:], -1.0)
                nmf = spool.tile([128, 1], F32, tag="nmf")
                nc.vector.tensor_copy(nmf[0:64, :], nmc[:])
                nc.vector.tensor_copy(nmf[64:128, :], nmc[:])
                en = spool.tile([128, NT], F32, tag="en")
                nc.scalar.activation(
                    en[:], attm[:], AF.Exp, bias=nmf[:], scale=1.0
                )
                s1 = spool.tile([128, 1], F32, tag="s1")
                nc.vector.reduce_sum(s1[:], en[:], axis=AX.X)
                s1u = spool.tile([64, 1], F32, tag="s1u")
                nc.vector.tensor_copy(s1u[:], s1[64:128, :])
                sc = spool.tile([64, 1], F32, tag="sc")
                nc.vector.tensor_tensor(sc[:], s1[0:64, :], s1u[:], op=ALU.add)
                rc = spool.tile([64, 1], F32, tag="rc")
                nc.vector.reciprocal(rc[:], sc[:])
                rf = spool.tile([128, 1], F32, tag="rf")
                nc.vector.tensor_copy(rf[0:64, :], rc[:])
                nc.vector.tensor_copy(rf[64:128, :], rc[:])
                snm = spool.tile([128, NT], F32, tag="snm")
                nc.vector.tensor_scalar_mul(snm[:], en[:], rf[:])

                # update WD ci columns (strided; padding pre-zeroed once)
                for j in range(2):
                    for g in range(2):
                        a0 = 4096 * g + j
                        nc.vector.tensor_copy(
                            WDf[64 * j : 64 * (j + 1), a0 : a0 + 130 * (TG - 1) + 1 : 130],
                            slT[64 * j : 64 * (j + 1), TG * g : TG * (g + 1)],
                        )

                # WD intra columns (dual-diag S_n stack) in one broadcast op
                i2b = (
                    i2_t[:]
                    .rearrange("p (a b l) -> p a b l", a=1, b=1)
                    .broadcast_to([128, 2, TG, 64])
                )
                snmb = (
                    snm[:]
                    .rearrange("p (g s o) -> p g s o", g=2, s=TG)
                    .broadcast_to([128, 2, TG, 64])
                )
                nc.vector.tensor_tensor(WDv[:, :, :, 64:128], i2b, snmb, op=ALU.mult)

            def p3_ci_intra(b):
                s = st[b]
                cth = s["cth"]
                # one matmul per tile computes both ci (rows 0-63, group
                # half of n) and the intra partial (rows 64-127)
                pgs = []
                for g in range(2):
                    ps_ = pap.tile([128, 256], F32, tag="acc")
                    pgs.append(ps_)
                    for ss_ in range(TG):
                        nc.tensor.matmul(
                            ps_[:],
                            WDf[:, 4096 * g + 128 * ss_ : 4096 * g + 128 * ss_ + 128],
                            cth[g][:, ss_, :],
                            start=(ss_ == 0),
                            stop=(ss_ == TG - 1),
                        )
                ciT = spool.tile([128, 256], FP16, tag="ciT")
                for g in range(2):
                    nc.vector.tensor_copy(ciT[64 * g : 64 * (g + 1), :], pgs[g][0:64, :])
                cta = spool.tile([64, 256], F32, tag="cta")
                nc.vector.tensor_copy(cta[:], pgs[0][64:128, :])
                ctT = spool.tile([64, 256], FP16, tag="ctT")
                nc.vector.tensor_tensor(ctT[:], cta[:], pgs[1][64:128, :], op=ALU.add)
                s["ciT"] = ciT
                s["ctT"] = ctT

            def p4_smalls(b):
                s = st[b]
                ciT, ctT = s["ciT"], s["ctT"]
                # a = wa @ curver_inter
                a_ps = psp.tile([128, 128], F32, tag="sm")
                for h in range(2):
                    tr_ps = ptp.tile([128, 128], FP16, tag="tps")
                    nc.tensor.transpose(
                        tr_ps[:], ciT[:, 128 * h : 128 * (h + 1)], eyeh_t[:]
                    )
                    cin = spool.tile([128, 128], FP16, tag="cin")
                    nc.vector.tensor_copy(cin[:], tr_ps[:])
                    nc.tensor.matmul(
                        a_ps[:], waT_c[h][:], cin[:],
                        start=(h == 0), stop=(h == 1),
                    )
                a_sb = apool.tile([MID, 128], FP16, tag="a_sb")
                nc.vector.tensor_copy(a_sb[:], a_ps[:])
                # b = wb @ curves_intra
                b_ps = psp.tile([128, 64], F32, tag="sm")
                for h in range(2):
                    tr2_ps = ptp.tile([128, 64], FP16, tag="tps")
                    nc.tensor.transpose(
                        tr2_ps[:], ctT[:, 128 * h : 128 * (h + 1)], eyeh_t[0:64, 0:64]
                    )
                    ctn = spool.tile([128, 64], FP16, tag="ctn")
                    nc.vector.tensor_copy(ctn[:], tr2_ps[:])
                    nc.tensor.matmul(
                        b_ps[:], wbT_c[h][:], ctn[:],
                        start=(h == 0), stop=(h == 1),
                    )
                b_sb = apool.tile([MID, 64], FP16, tag="b_sb")
                nc.vector.tensor_copy(b_sb[:], b_ps[:])
                # AiT / AtT (wc.T @ a, wc.T @ b)
                AiT = []
                AtT = []
                for cc in range(2):
                    ai_ps = psp.tile([128, 128], F32, tag="sm")
                    nc.tensor.matmul(
                        ai_ps[:], wc_t[:, 128 * cc : 128 * (cc + 1)], a_sb[:],
                        start=True, stop=True,
                    )
                    ai = apool.tile([128, 128], FP16, tag="ai")
                    nc.vector.tensor_copy(ai[:], ai_ps[:])
                    AiT.append(ai)
                    at_ps = psp.tile([128, 64], F32, tag="sm")
                    nc.tensor.matmul(
                        at_ps[:], wc_t[:, 128 * cc : 128 * (cc + 1)], b_sb[:],
                        start=True, stop=True,
                    )
                    at = apool.tile([128, 64], FP16, tag="at")
                    nc.vector.tensor_copy(at[:], at_ps[:])
                    AtT.append(at)
                # UiT / UtT
                ui_ps = psp.tile([128, 256], F32, tag="sm")
                nc.tensor.matmul(ui_ps[:], a_sb[:], wdnT_t[:], start=True, stop=True)
                UiT = apool.tile([128, 256], BF, tag="UiT")
                nc.vector.tensor_copy(UiT[:], ui_ps[:])
                ut_ps = psp.tile([64, 256], F32, tag="sm")
                nc.tensor.matmul(ut_ps[:], b_sb[:], wdlT_t[:], start=True, stop=True)
                UtT = apool.tile([64, 256], BF, tag="UtT")
                nc.vector.tensor_copy(UtT[:], ut_ps[:])
                s["AiT"], s["AtT"], s["UiT"], s["UtT"] = AiT, AtT, UiT, UtT

            def p5_pass_a(b):
                s = st[b]
                AiT, AtT = s["AiT"], s["AtT"]
                ei_all = epool.tile([128, NCH, CHW], BF, tag="ei")
                et_all = epool.tile([64, NCH, CHW], BF, tag="et")
                xt = [[None] * NJ for _ in range(2)]
                for j in range(NJ):
                    for cc in range(2):
                        t_ = xpool.tile([128, XJW], FP16, tag="xin")
                        nc.sync.dma_start(
                            t_[:],
                            x_d[b, 128 * cc : 128 * (cc + 1), XJW * j : XJW * (j + 1)],
                        )
                        xt[cc][j] = t_
                    for kk in range(XJW // CHW):
                        k = j * (XJW // CHW) + kk
                        xs = [
                            xt[cc][j][:, CHW * kk : CHW * (kk + 1)] for cc in range(2)
                        ]
                        pi = plp.tile([128, CHW], F32, tag="pl")
                        for cc in range(2):
                            nc.tensor.matmul(
                                pi[:], AiT[cc][:], xs[cc],
                                start=(cc == 0), stop=(cc == 1),
                            )
                        nc.scalar.activation(ei_all[:, k, :], pi[:], AF.Exp)
                        pt = plp.tile([64, CHW], F32, tag="pl")
                        for cc in range(2):
                            nc.tensor.matmul(
                                pt[:], AtT[cc][:], xs[cc],
                                start=(cc == 0), stop=(cc == 1),
                            )
                        nc.scalar.activation(et_all[:, k, :], pt[:], AF.Exp)
                s["ei"], s["et"], s["xt"] = ei_all, et_all, xt

            QC = XJW // CHW   # chunks per quarter (= per y staging group)

            def p6_dens(b, q):
                """Pass B quarter q: denominator matmuls + DVE approx recips
                + in-place normalize of this quarter of E."""
                s = st[b]
                ei_all, et_all = s["ei"], s["et"]
                riq = rpool.tile([128, QC, CHW], BF, tag="riq")
                rtq = rpool.tile([64, QC, CHW], BF, tag="rtq")
                for kk in range(QC):
                    k = q * QC + kk
                    di = plp.tile([128, CHW], F32, tag="pl")
                    nc.tensor.matmul(
                        di[:], onesn_t[:], ei_all[:, k, :], start=True, stop=True
                    )
                    _act_recip(nc, riq[:, kk, :], di[:])
                    dt_ = plp.tile([64, CHW], F32, tag="pl")
                    nc.tensor.matmul(
                        dt_[:], onesl_t[:], et_all[:, k, :], start=True, stop=True
                    )
                    _act_recip(nc, rtq[:, kk, :], dt_[:])
                k0, k1 = q * QC, (q + 1) * QC
                nc.vector.tensor_tensor(
                    ei_all[:, k0:k1, :], ei_all[:, k0:k1, :], riq[:], op=ALU.mult
                )
                nc.vector.tensor_tensor(
                    et_all[:, k0:k1, :], et_all[:, k0:k1, :], rtq[:], op=ALU.mult
                )

            def p7_y(b, q):
                """Pass B quarter q: y matmuls (x folded via identity) + ACT
                Lrelu epilogue (beta via bias) + staged store."""
                s = st[b]
                ei_all, et_all, xt = s["ei"], s["et"], s["xt"]
                UiT, UtT = s["UiT"], s["UtT"]
                j = q
                ost = []
                for cc in range(2):
                    ostt = opool.tile([128, XJW], FP16, tag="ost")
                    ost.append(ostt)
                for kk in range(QC):
                    k = j * QC + kk
                    for cc in range(2):
                        yps = pyp.tile([128, CHW], F32, tag="yps")
                        nc.tensor.matmul(
                            yps[:],
                            UiT[:, 128 * cc : 128 * (cc + 1)],
                            ei_all[:, k, :],
                            start=True, stop=False,
                        )
                        nc.tensor.matmul(
                            yps[:],
                            UtT[:, 128 * cc : 128 * (cc + 1)],
                            et_all[:, k, :],
                            start=False, stop=True,
                        )
                        # t1 = yps + beta + x, straight into the staging tile
                        nc.vector.scalar_tensor_tensor(
                            ost[cc][:, CHW * kk : CHW * (kk + 1)],
                            yps[:], betaps[cc][:],
                            xt[cc][j][:, CHW * kk : CHW * (kk + 1)],
                            op0=ALU.add, op1=ALU.add,
                        )
                for cc in range(2):
                    # bulk leaky-relu in place (fp16 SBUF -> DVE 2x mode)
                    nc.vector.scalar_tensor_tensor(
                        ost[cc][:], ost[cc][:], 0.2, ost[cc][:],
                        op0=ALU.mult, op1=ALU.max,
                    )
                    nc.sync.dma_start(
                        y_d[b, 128 * cc : 128 * (cc + 1), XJW * j : XJW * (j + 1)],
                        ost[cc][:],
                    )

            def pass_b(b):
                for q in range(NJ):
                    p6_dens(b, q)
                    if q >= 1:
                        p7_y(b, q - 1)
                p7_y(b, NJ - 1)

            # ---- interleaved emission: keep the PE queue dense ----
            p1_load_att(0)
            p1_load_att(1)
            p2_softmax(0)
            p3_ci_intra(0)
            p2_softmax(1)
            p4_smalls(0)
            p3_ci_intra(1)
            p5_pass_a(0)
            p4_smalls(1)
            pass_b(0)
            p5_pass_a(1)
            pass_b(1)

    _split_excess_waits(nc, max_waits=1)
    return nc


_CACHE = {}


def _get_bass():
    if "nc" not in _CACHE:
        _CACHE["nc"] = _build_bass()
    return _CACHE["nc"]


def kernel(x, curves, w_att, wa, wb, wc, wn, wl, wd,
           bn_gamma, bn_beta, bn_mean, bn_var):
    x = np.asarray(x, dtype=np.float32)
    curves = np.asarray(curves, dtype=np.float32)
    w_att = np.asarray(w_att, dtype=np.float32)
    wa = np.asarray(wa, dtype=np.float32)
    wb = np.asarray(wb, dtype=np.float32)
    wc = np.asarray(wc, dtype=np.float32)
    wn = np.asarray(wn, dtype=np.float32)
    wl = np.asarray(wl, dtype=np.float32)
    wd = np.asarray(wd, dtype=np.float32)
    bn_gamma = np.asarray(bn_gamma, dtype=np.float32)
    bn_beta = np.asarray(bn_beta, dtype=np.float32)
    bn_mean = np.asarray(bn_mean, dtype=np.float32)
    bn_var = np.asarray(bn_var, dtype=np.float32)

    s = bn_gamma / np.sqrt(bn_var + BN_EPS)
    betap = (bn_beta - bn_mean * s).astype(np.float32).reshape(C, 1)
    wdnT = ((wd[:, :MID] @ wn).T * s[None, :]).astype(np.float16)
    wdlT = ((wd[:, MID:] @ wl).T * s[None, :]).astype(np.float16)

    consts = {
        "wc": np.ascontiguousarray(wc).astype(np.float16),
        "waT": np.ascontiguousarray(wa.T).astype(np.float16),
        "wbT": np.ascontiguousarray(wb.T).astype(np.float16),
        "wdnT": np.ascontiguousarray(wdnT),
        "wdlT": np.ascontiguousarray(wdlT),
        "wattT": np.ascontiguousarray(
            np.broadcast_to(w_att.reshape(C, 1), (C, 128))
        ).astype(np.float16),
        "onesn": np.ones((128, 128), bf16),
        "onesl": np.ones((64, 64), bf16),
        "i2": np.concatenate([np.eye(64), np.eye(64)], axis=0).astype(np.float16),
        "eyeh": np.eye(128).astype(np.float16),
        "betap": betap,
    }

    x16 = x.astype(np.float16)                      # [B, C, N]
    cn16 = curves.reshape(B, C, CN * CL).astype(np.float16)
    # cT[q, t, c] = curves[c, 128 t + q] (host-side transpose)
    ct16 = np.ascontiguousarray(
        cn16.reshape(B, C, NT, 128).transpose(0, 3, 2, 1).reshape(B, 128, NT * 256)
    )

    in_maps = []
    for core in range(NCORES):
        b0 = core * NB
        m = dict(consts)
        m["x"] = np.ascontiguousarray(x16[b0 : b0 + NB])
        m["cnat"] = np.ascontiguousarray(cn16[b0 : b0 + NB])
        m["ctr"] = np.ascontiguousarray(ct16[b0 : b0 + NB])
        in_maps.append(m)

    nc = _get_bass()
    res = run_bass_kernel_spmd(nc, in_maps, core_ids=list(range(NCORES)))
    out = np.empty((B, C, N), np.float32)
    for core in range(NCORES):
        out[core * NB : (core + 1) * NB] = res.results[core]["y"].astype(np.float32)
    return out
